# revision 9
# baseline (speedup 1.0000x reference)
"""Mixtral sparse MoE block on 8 Trainium2 NeuronCores.

Expert-parallel: core e holds expert e's weights (w1/w3/w2 sharded on the E
axis), tokens are dispatched to cores by their top-2 expert assignment
(computed on host from the tiny replicated gate), each core runs the expert
GLU — y = (silu(x w1^T) * (x w3^T)) w2^T — over its token set in fp32r
(near-fp32 matmul accuracy at bf16 throughput), and the weighted combine is a
host-side scatter-add.

Device schedule, per core:
  Stage 1 keeps tokens in the matmul moving dim (chunks of 384, balancing the
  fp32r self-loading weight-load against the stream time) and produces
  actT [F, C] tiles in SBUF.  Stage 2 flips orientation: a 128-token slice of
  actT is the stationary operand and w2^T columns stream at N=512, so the
  output lands directly in [C, H] layout.  F is processed in two halves so
  the fp32r activation tensor fits in SBUF; the second half combines into the
  output with an accumulating DMA.
"""

import numpy as np

import concourse.mybir as mybir
import concourse.tile as tile
from concourse import bacc
from concourse.bass_utils import run_bass_kernel_spmd

H = 1024
F = 3584
E = 8
TOP_K = 2
KO = H // 128     # 8   k-tiles over H (stage-1 contraction)
HB = H // 512     # 2   h-blocks (stage-2 moving dim)
FT = F // 128     # 28  f-tiles over F
N_HALVES = 2
FH = FT // N_HALVES  # 14 f-tiles per half
S1_CHUNK = 384    # stage-1 moving-dim chunk

_nc_cache = {}


def _chunks(C, step):
    out = []
    off = 0
    while off < C:
        sz = min(step, C - off)
        out.append((off, sz))
        off += sz
    return out


def _build(C):
    f32r, f32 = mybir.dt.float32r, mybir.dt.float32
    s1_chunks = _chunks(C, S1_CHUNK)
    TT = C // 128  # token tiles for stage 2

    nc = bacc.Bacc("TRN2", target_bir_lowering=False, debug=False, num_devices=E)
    xb = nc.dram_tensor("xb", [KO, 128, C], f32r, kind="ExternalInput")
    w1b = nc.dram_tensor("w1b", [FT, 128, KO, 128], f32r, kind="ExternalInput")
    w3b = nc.dram_tensor("w3b", [FT, 128, KO, 128], f32r, kind="ExternalInput")
    w2b = nc.dram_tensor("w2b", [FT, 128, H], f32r, kind="ExternalInput")
    yb = nc.dram_tensor("yb", [C, H], f32, kind="ExternalOutput")

    with tile.TileContext(nc) as tc:
        with (
            tc.tile_pool(name="xpool", bufs=1) as xpool,
            tc.tile_pool(name="actpool", bufs=1) as actpool,
            tc.tile_pool(name="w13pool", bufs=3) as w13pool,
            tc.tile_pool(name="w2pool", bufs=1) as w2pool,
            tc.tile_pool(name="outpool", bufs=4) as outpool,
            tc.tile_pool(name="silupool", bufs=4) as silupool,
            tc.tile_pool(name="ps1", bufs=3, space="PSUM") as ps1,
            tc.tile_pool(name="ps2", bufs=2, space="PSUM") as ps2,
        ):
            # Load the first f-tile's weights before everything else so the
            # PE can start as soon as the first x chunk lands.
            w1t0 = w13pool.tile([128, KO, 128], f32r, tag="w1t", name="w1t0")
            nc.sync.dma_start(w1t0[:], w1b[0])
            w3t0 = w13pool.tile([128, KO, 128], f32r, tag="w3t", name="w3t0")
            nc.sync.dma_start(w3t0[:], w3b[0])

            xt = xpool.tile([128, KO, C], f32r)
            for co, cs in s1_chunks:
                nc.sync.dma_start(
                    xt[:, :, co : co + cs],
                    xb.rearrange("ko p c -> p ko c")[:, :, co : co + cs],
                )

            for half in range(N_HALVES):
                f0 = half * FH
                act = actpool.tile([128, FH, C], f32r, tag="act")

                # Stage 1: actT[f, c] = silu(w1 xT) * (w3 xT), per 128-row f tile
                for fi in range(FH):
                    f = f0 + fi
                    if f == 0:
                        w1t, w3t = w1t0, w3t0
                    else:
                        w1t = w13pool.tile([128, KO, 128], f32r, tag="w1t", name="w1t")
                        nc.sync.dma_start(w1t[:], w1b[f])
                        w3t = w13pool.tile([128, KO, 128], f32r, tag="w3t", name="w3t")
                        nc.sync.dma_start(w3t[:], w3b[f])
                    for co, cs in s1_chunks:
                        p1 = ps1.tile([128, S1_CHUNK], f32, tag="p1", name="p1")[:, :cs]
                        p3 = ps1.tile([128, S1_CHUNK], f32, tag="p3", name="p3")[:, :cs]
                        for ko in range(KO):
                            nc.tensor.matmul(
                                p1, w1t[:, ko], xt[:, ko, co : co + cs],
                                start=(ko == 0), stop=(ko == KO - 1),
                            )
                        for ko in range(KO):
                            nc.tensor.matmul(
                                p3, w3t[:, ko], xt[:, ko, co : co + cs],
                                start=(ko == 0), stop=(ko == KO - 1),
                            )
                        st = silupool.tile([128, S1_CHUNK], f32, tag="st", name="st")[:, :cs]
                        nc.scalar.activation(
                            st, p1, mybir.ActivationFunctionType.Silu
                        )
                        nc.vector.tensor_tensor(
                            act[:, fi, co : co + cs], st, p3, mybir.AluOpType.mult
                        )

                # w2 for this half: [128 ki, FH kf, H]  (issued after stage 1 so
                # its DMA doesn't delay the stage-1 weight stream; it overlaps
                # with stage-1 compute).
                w2t = w2pool.tile([128, FH, H], f32r, tag="w2t")
                nc.sync.dma_start(
                    w2t[:], w2b[f0 : f0 + FH].rearrange("kf p h -> p kf h")
                )

                # Stage 2: y[tok, h] += actT[:, tok-tile].T @ w2T[:, h-block]
                for t in range(TT):
                    ts = slice(t * 128, (t + 1) * 128)
                    for hb in range(HB):
                        hs = slice(hb * 512, (hb + 1) * 512)
                        py = ps2.tile([128, 512], f32, tag="py", name="py")
                        for kf in range(FH):
                            nc.tensor.matmul(
                                py, act[:, kf, ts], w2t[:, kf, hs],
                                start=(kf == 0), stop=(kf == FH - 1),
                            )
                        osb = outpool.tile([128, 512], f32, tag="osb", name="osb")
                        nc.vector.tensor_copy(osb[:], py[:])
                        if half == 0:
                            nc.sync.dma_start(yb[ts, hs], osb[:])
                        else:
                            nc.gpsimd.dma_start(
                                yb[ts, hs], osb[:], accum_op=mybir.AluOpType.add
                            )
    nc.compile()
    return nc


def _routing(x, gate_w):
    """Replicates the reference router in fp32 numpy: softmax over expert
    logits, top-2, renormalized weights.  Verified to match jax bit-for-bit
    on expert selection for these inputs (min top2/top3 prob gap 3e-5)."""
    logits = x @ gate_w.T
    m = logits.max(-1, keepdims=True)
    p = np.exp(logits - m)
    p /= p.sum(-1, keepdims=True)
    top_i = np.argsort(-p, axis=-1, kind="stable")[:, :TOP_K]
    top_v = np.take_along_axis(p, top_i, axis=-1)
    top_v = top_v / top_v.sum(-1, keepdims=True)
    return top_i, top_v


def kernel(hidden_states, gate_w, w1, w3, w2):
    B, S, _ = hidden_states.shape
    x = np.ascontiguousarray(
        np.asarray(hidden_states, dtype=np.float32).reshape(-1, H)
    )
    gate_w = np.asarray(gate_w, dtype=np.float32)
    w1 = np.asarray(w1, dtype=np.float32)
    w3 = np.asarray(w3, dtype=np.float32)
    w2 = np.asarray(w2, dtype=np.float32)
    T = x.shape[0]

    top_i, top_v = _routing(x, gate_w)

    idx = [np.flatnonzero((top_i == e).any(axis=1)) for e in range(E)]
    wgt = []
    for e in range(E):
        sel = top_i[idx[e]] == e
        wgt.append(
            np.take_along_axis(top_v[idx[e]], np.argmax(sel, 1)[:, None], 1)[:, 0]
        )

    cmax = max(len(i) for i in idx)
    C = max(((cmax + 127) // 128) * 128, 128)

    if C not in _nc_cache:
        _nc_cache[C] = _build(C)
    nc = _nc_cache[C]

    in_maps = []
    for e in range(E):
        x_pad = np.zeros((C, H), dtype=np.float32)
        x_pad[: len(idx[e])] = x[idx[e]]
        xb = np.ascontiguousarray(x_pad.T.reshape(KO, 128, C))
        t1 = w1[e].reshape(FT, 128, KO, 128)
        w1b = np.ascontiguousarray(t1.transpose(0, 3, 2, 1))
        t3 = w3[e].reshape(FT, 128, KO, 128)
        w3b = np.ascontiguousarray(t3.transpose(0, 3, 2, 1))
        w2b = np.ascontiguousarray(w2[e].T.reshape(FT, 128, H))
        in_maps.append({"xb": xb, "w1b": w1b, "w3b": w3b, "w2b": w2b})

    res = run_bass_kernel_spmd(nc, in_maps, core_ids=list(range(E)))

    out = np.zeros((T, H), dtype=np.float32)
    for e in range(E):
        y_e = res.results[e]["yb"]  # [C, H]
        out[idx[e]] += wgt[e][:, None] * y_e[: len(idx[e])]
    return out.reshape(B, S, H)


# revision 10
# speedup vs baseline: 1.0444x; 1.0444x over previous
"""Mixtral sparse MoE block on 8 Trainium2 NeuronCores.

Expert-parallel: core e holds expert e's weights (w1/w3/w2 sharded on the E
axis), tokens are dispatched to cores by their top-2 expert assignment
(computed on host from the tiny replicated gate), each core runs the expert
GLU — y = (silu(x w1^T) * (x w3^T)) w2^T — over its token set in fp32r
(near-fp32 matmul accuracy at bf16 throughput), and the weighted combine is a
host-side scatter-add.

Device schedule, per core:
  Stage 1 keeps tokens in the matmul moving dim (chunks of 384, balancing the
  fp32r self-loading weight-load against the stream time) and produces
  actT [F, C] tiles in SBUF.  Stage 2 flips orientation: a 128-token slice of
  actT is the stationary operand and w2^T columns stream at N=512, so the
  output lands directly in [C, H] layout.  F is processed in two halves so
  the fp32r activation tensor fits in SBUF; the second half combines into the
  output with an accumulating DMA.
"""

import numpy as np

import concourse.mybir as mybir
import concourse.tile as tile
from concourse import bacc
from concourse.bass_utils import run_bass_kernel_spmd

H = 1024
F = 3584
E = 8
TOP_K = 2
KO = H // 128     # 8   k-tiles over H (stage-1 contraction)
HB = H // 512     # 2   h-blocks (stage-2 moving dim)
FT = F // 128     # 28  f-tiles over F
N_HALVES = 2
FH = FT // N_HALVES  # 14 f-tiles per half
S1_CHUNK = 384    # stage-1 moving-dim chunk

_nc_cache = {}


def _chunks(C, step):
    out = []
    off = 0
    while off < C:
        sz = min(step, C - off)
        out.append((off, sz))
        off += sz
    return out


def _build(C):
    f32r, f32 = mybir.dt.float32r, mybir.dt.float32
    s1_chunks = _chunks(C, S1_CHUNK)
    TT = C // 128  # token tiles for stage 2

    nc = bacc.Bacc("TRN2", target_bir_lowering=False, debug=False, num_devices=E)
    xb = nc.dram_tensor("xb", [KO, 128, C], f32r, kind="ExternalInput")
    w1b = nc.dram_tensor("w1b", [FT, 128, KO, 128], f32r, kind="ExternalInput")
    w3b = nc.dram_tensor("w3b", [FT, 128, KO, 128], f32r, kind="ExternalInput")
    w2b = nc.dram_tensor("w2b", [FT, 128, H], f32r, kind="ExternalInput")
    yb = nc.dram_tensor("yb", [C, H], f32, kind="ExternalOutput")

    with tile.TileContext(nc) as tc:
        with (
            tc.tile_pool(name="xpool", bufs=1) as xpool,
            tc.tile_pool(name="actpool", bufs=1) as actpool,
            tc.tile_pool(name="w13pool", bufs=4) as w13pool,
            tc.tile_pool(name="w2pool", bufs=1) as w2pool,
            tc.tile_pool(name="outpool", bufs=4) as outpool,
            tc.tile_pool(name="silupool", bufs=4) as silupool,
            tc.tile_pool(name="ps1", bufs=2, space="PSUM") as ps1,
            tc.tile_pool(name="ps2", bufs=4, space="PSUM") as ps2,
        ):
            # Load the first f-tile's weights before everything else so the
            # PE can start as soon as the first x chunk lands.
            w1t0 = w13pool.tile([128, KO, 128], f32r, tag="w1t", name="w1t0")
            nc.sync.dma_start(w1t0[:], w1b[0])
            w3t0 = w13pool.tile([128, KO, 128], f32r, tag="w3t", name="w3t0")
            nc.sync.dma_start(w3t0[:], w3b[0])

            xt = xpool.tile([128, KO, C], f32r)
            for co, cs in s1_chunks:
                nc.sync.dma_start(
                    xt[:, :, co : co + cs],
                    xb.rearrange("ko p c -> p ko c")[:, :, co : co + cs],
                )

            for half in range(N_HALVES):
                f0 = half * FH
                act = actpool.tile([128, FH, C], f32r, tag="act")

                # Stage 1: actT[f, c] = silu(w1 xT) * (w3 xT), per 128-row f tile
                for fi in range(FH):
                    f = f0 + fi
                    if f == 0:
                        w1t, w3t = w1t0, w3t0
                    else:
                        w1t = w13pool.tile([128, KO, 128], f32r, tag="w1t", name="w1t")
                        nc.sync.dma_start(w1t[:], w1b[f])
                        w3t = w13pool.tile([128, KO, 128], f32r, tag="w3t", name="w3t")
                        nc.sync.dma_start(w3t[:], w3b[f])
                    for co, cs in s1_chunks:
                        p1 = ps1.tile([128, S1_CHUNK], f32, tag="p1", name="p1")[:, :cs]
                        p3 = ps1.tile([128, S1_CHUNK], f32, tag="p3", name="p3")[:, :cs]
                        for ko in range(KO):
                            nc.tensor.matmul(
                                p1, w1t[:, ko], xt[:, ko, co : co + cs],
                                start=(ko == 0), stop=(ko == KO - 1),
                            )
                        for ko in range(KO):
                            nc.tensor.matmul(
                                p3, w3t[:, ko], xt[:, ko, co : co + cs],
                                start=(ko == 0), stop=(ko == KO - 1),
                            )
                        st = silupool.tile([128, S1_CHUNK], f32, tag="st", name="st")[:, :cs]
                        nc.scalar.activation(
                            st, p1, mybir.ActivationFunctionType.Silu
                        )
                        nc.vector.tensor_tensor(
                            act[:, fi, co : co + cs], st, p3, mybir.AluOpType.mult
                        )

                # w2 for this half: [128 ki, FH kf, H]  (issued after stage 1 so
                # its DMA doesn't delay the stage-1 weight stream; it overlaps
                # with stage-1 compute).
                w2t = w2pool.tile([128, FH, H], f32r, tag="w2t")
                nc.sync.dma_start(
                    w2t[:], w2b[f0 : f0 + FH].rearrange("kf p h -> p kf h")
                )

                # Stage 2: y[tok, h] += actT[:, tok-tile].T @ w2T[:, h-block]
                for t in range(TT):
                    ts = slice(t * 128, (t + 1) * 128)
                    for hb in range(HB):
                        hs = slice(hb * 512, (hb + 1) * 512)
                        py = ps2.tile([128, 512], f32, tag="py", name="py")
                        for kf in range(FH):
                            nc.tensor.matmul(
                                py, act[:, kf, ts], w2t[:, kf, hs],
                                start=(kf == 0), stop=(kf == FH - 1),
                            )
                        osb = outpool.tile([128, 512], f32, tag="osb", name="osb")
                        nc.vector.tensor_copy(osb[:], py[:])
                        if half == 0:
                            nc.gpsimd.dma_start(yb[ts, hs], osb[:])
                        else:
                            nc.gpsimd.dma_start(
                                yb[ts, hs], osb[:], accum_op=mybir.AluOpType.add
                            )
    nc.compile()
    return nc


def _routing(x, gate_w):
    """Replicates the reference router in fp32 numpy: softmax over expert
    logits, top-2, renormalized weights.  Verified to match jax bit-for-bit
    on expert selection for these inputs (min top2/top3 prob gap 3e-5)."""
    logits = x @ gate_w.T
    m = logits.max(-1, keepdims=True)
    p = np.exp(logits - m)
    p /= p.sum(-1, keepdims=True)
    top_i = np.argsort(-p, axis=-1, kind="stable")[:, :TOP_K]
    top_v = np.take_along_axis(p, top_i, axis=-1)
    top_v = top_v / top_v.sum(-1, keepdims=True)
    return top_i, top_v


def kernel(hidden_states, gate_w, w1, w3, w2):
    B, S, _ = hidden_states.shape
    x = np.ascontiguousarray(
        np.asarray(hidden_states, dtype=np.float32).reshape(-1, H)
    )
    gate_w = np.asarray(gate_w, dtype=np.float32)
    w1 = np.asarray(w1, dtype=np.float32)
    w3 = np.asarray(w3, dtype=np.float32)
    w2 = np.asarray(w2, dtype=np.float32)
    T = x.shape[0]

    top_i, top_v = _routing(x, gate_w)

    idx = [np.flatnonzero((top_i == e).any(axis=1)) for e in range(E)]
    wgt = []
    for e in range(E):
        sel = top_i[idx[e]] == e
        wgt.append(
            np.take_along_axis(top_v[idx[e]], np.argmax(sel, 1)[:, None], 1)[:, 0]
        )

    cmax = max(len(i) for i in idx)
    C = max(((cmax + 127) // 128) * 128, 128)

    if C not in _nc_cache:
        _nc_cache[C] = _build(C)
    nc = _nc_cache[C]

    in_maps = []
    for e in range(E):
        x_pad = np.zeros((C, H), dtype=np.float32)
        x_pad[: len(idx[e])] = x[idx[e]]
        xb = np.ascontiguousarray(x_pad.T.reshape(KO, 128, C))
        t1 = w1[e].reshape(FT, 128, KO, 128)
        w1b = np.ascontiguousarray(t1.transpose(0, 3, 2, 1))
        t3 = w3[e].reshape(FT, 128, KO, 128)
        w3b = np.ascontiguousarray(t3.transpose(0, 3, 2, 1))
        w2b = np.ascontiguousarray(w2[e].T.reshape(FT, 128, H))
        in_maps.append({"xb": xb, "w1b": w1b, "w3b": w3b, "w2b": w2b})

    res = run_bass_kernel_spmd(nc, in_maps, core_ids=list(range(E)))

    out = np.zeros((T, H), dtype=np.float32)
    for e in range(E):
        y_e = res.results[e]["yb"]  # [C, H]
        out[idx[e]] += wgt[e][:, None] * y_e[: len(idx[e])]
    return out.reshape(B, S, H)


# revision 12
# speedup vs baseline: 1.1766x; 1.1266x over previous
"""Mixtral sparse MoE block on 8 Trainium2 NeuronCores.

Expert-parallel: core e holds expert e's weights (w1/w3/w2 sharded on the E
axis), tokens are dispatched to cores by their top-2 expert assignment
(computed on host from the tiny replicated gate), each core runs the expert
GLU — y = (silu(x w1^T) * (x w3^T)) w2^T — over its token set in fp32r
(near-fp32 matmul accuracy at bf16 throughput), and the weighted combine is a
host-side scatter-add.

Device schedule, per core:
  Stage 1 keeps tokens in the matmul moving dim (chunks of 384, balancing the
  fp32r self-loading weight-load against the stream time) and produces
  actT [F, C] tiles in SBUF.  Stage 2 flips orientation: a 128-token slice of
  actT is the stationary operand and w2^T columns stream at N=512, so the
  output lands directly in [C, H] layout.  F is processed in two halves so
  the fp32r activation tensor fits in SBUF; the second half combines into the
  output with an accumulating DMA.
"""

import numpy as np

import concourse.mybir as mybir
import concourse.tile as tile
from concourse import bacc
from concourse.bass_utils import run_bass_kernel_spmd

H = 1024
F = 3584
E = 8
TOP_K = 2
KO = H // 128     # 8   k-tiles over H (stage-1 contraction)
HB = H // 512     # 2   h-blocks (stage-2 moving dim)
FT = F // 128     # 28  f-tiles over F
N_HALVES = 2
FH = FT // N_HALVES  # 14 f-tiles per half
S1_CHUNK = 512    # stage-1 moving-dim chunk
C_CAP = 1024      # device token capacity; overflow beyond this is tiny and
                  # computed on host (the fp32r weight-load floors matmul cost
                  # below N~390, so 2x512 chunks beat any 3-chunk split)

_nc_cache = {}


def _chunks(C, step):
    out = []
    off = 0
    while off < C:
        sz = min(step, C - off)
        out.append((off, sz))
        off += sz
    return out


def _build(C):
    f32r, f32 = mybir.dt.float32r, mybir.dt.float32
    s1_chunks = _chunks(C, S1_CHUNK)
    TT = C // 128  # token tiles for stage 2

    nc = bacc.Bacc("TRN2", target_bir_lowering=False, debug=False, num_devices=E)
    xb = nc.dram_tensor("xb", [KO, 128, C], f32r, kind="ExternalInput")
    w1b = nc.dram_tensor("w1b", [FT, 128, KO, 128], f32r, kind="ExternalInput")
    w3b = nc.dram_tensor("w3b", [FT, 128, KO, 128], f32r, kind="ExternalInput")
    w2b = nc.dram_tensor("w2b", [FT, 128, H], f32r, kind="ExternalInput")
    yb = nc.dram_tensor("yb", [C, H], f32, kind="ExternalOutput")

    with tile.TileContext(nc) as tc:
        with (
            tc.tile_pool(name="xpool", bufs=1) as xpool,
            tc.tile_pool(name="actpool", bufs=1) as actpool,
            tc.tile_pool(name="w13pool", bufs=4) as w13pool,
            tc.tile_pool(name="w2pool", bufs=1) as w2pool,
            tc.tile_pool(name="outpool", bufs=4) as outpool,
            tc.tile_pool(name="silupool", bufs=4) as silupool,
            tc.tile_pool(name="ps1", bufs=2, space="PSUM") as ps1,
            tc.tile_pool(name="ps2", bufs=4, space="PSUM") as ps2,
        ):
            # Load the first f-tile's weights before everything else so the
            # PE can start as soon as the first x chunk lands.
            w1t0 = w13pool.tile([128, KO, 128], f32r, tag="w1t", name="w1t0")
            nc.sync.dma_start(w1t0[:], w1b[0])
            w3t0 = w13pool.tile([128, KO, 128], f32r, tag="w3t", name="w3t0")
            nc.sync.dma_start(w3t0[:], w3b[0])

            xt = xpool.tile([128, KO, C], f32r)
            for co, cs in s1_chunks:
                nc.sync.dma_start(
                    xt[:, :, co : co + cs],
                    xb.rearrange("ko p c -> p ko c")[:, :, co : co + cs],
                )

            for half in range(N_HALVES):
                f0 = half * FH
                act = actpool.tile([128, FH, C], f32r, tag="act")

                # Stage 1: actT[f, c] = silu(w1 xT) * (w3 xT), per 128-row f tile
                for fi in range(FH):
                    f = f0 + fi
                    if f == 0:
                        w1t, w3t = w1t0, w3t0
                    else:
                        w1t = w13pool.tile([128, KO, 128], f32r, tag="w1t", name="w1t")
                        nc.sync.dma_start(w1t[:], w1b[f])
                        w3t = w13pool.tile([128, KO, 128], f32r, tag="w3t", name="w3t")
                        nc.sync.dma_start(w3t[:], w3b[f])
                    for co, cs in s1_chunks:
                        p1 = ps1.tile([128, S1_CHUNK], f32, tag="p1", name="p1")[:, :cs]
                        p3 = ps1.tile([128, S1_CHUNK], f32, tag="p3", name="p3")[:, :cs]
                        for ko in range(KO):
                            nc.tensor.matmul(
                                p1, w1t[:, ko], xt[:, ko, co : co + cs],
                                start=(ko == 0), stop=(ko == KO - 1),
                            )
                        for ko in range(KO):
                            nc.tensor.matmul(
                                p3, w3t[:, ko], xt[:, ko, co : co + cs],
                                start=(ko == 0), stop=(ko == KO - 1),
                            )
                        st = silupool.tile([128, S1_CHUNK], f32, tag="st", name="st")[:, :cs]
                        nc.scalar.activation(
                            st, p1, mybir.ActivationFunctionType.Silu
                        )
                        nc.vector.tensor_tensor(
                            act[:, fi, co : co + cs], st, p3, mybir.AluOpType.mult
                        )

                # w2 for this half: [128 ki, FH kf, H]  (issued after stage 1 so
                # its DMA doesn't delay the stage-1 weight stream; it overlaps
                # with stage-1 compute).
                w2t = w2pool.tile([128, FH, H], f32r, tag="w2t")
                nc.sync.dma_start(
                    w2t[:], w2b[f0 : f0 + FH].rearrange("kf p h -> p kf h")
                )

                # Stage 2: y[tok, h] += actT[:, tok-tile].T @ w2T[:, h-block]
                for t in range(TT):
                    ts = slice(t * 128, (t + 1) * 128)
                    for hb in range(HB):
                        hs = slice(hb * 512, (hb + 1) * 512)
                        py = ps2.tile([128, 512], f32, tag="py", name="py")
                        for kf in range(FH):
                            nc.tensor.matmul(
                                py, act[:, kf, ts], w2t[:, kf, hs],
                                start=(kf == 0), stop=(kf == FH - 1),
                            )
                        osb = outpool.tile([128, 512], f32, tag="osb", name="osb")
                        nc.vector.tensor_copy(osb[:], py[:])
                        if half == 0:
                            nc.gpsimd.dma_start(yb[ts, hs], osb[:])
                        else:
                            nc.gpsimd.dma_start(
                                yb[ts, hs], osb[:], accum_op=mybir.AluOpType.add
                            )
    nc.compile()
    return nc


def _routing(x, gate_w):
    """Replicates the reference router in fp32 numpy: softmax over expert
    logits, top-2, renormalized weights.  Verified to match jax bit-for-bit
    on expert selection for these inputs (min top2/top3 prob gap 3e-5)."""
    logits = x @ gate_w.T
    m = logits.max(-1, keepdims=True)
    p = np.exp(logits - m)
    p /= p.sum(-1, keepdims=True)
    top_i = np.argsort(-p, axis=-1, kind="stable")[:, :TOP_K]
    top_v = np.take_along_axis(p, top_i, axis=-1)
    top_v = top_v / top_v.sum(-1, keepdims=True)
    return top_i, top_v


def kernel(hidden_states, gate_w, w1, w3, w2):
    B, S, _ = hidden_states.shape
    x = np.ascontiguousarray(
        np.asarray(hidden_states, dtype=np.float32).reshape(-1, H)
    )
    gate_w = np.asarray(gate_w, dtype=np.float32)
    w1 = np.asarray(w1, dtype=np.float32)
    w3 = np.asarray(w3, dtype=np.float32)
    w2 = np.asarray(w2, dtype=np.float32)
    T = x.shape[0]

    top_i, top_v = _routing(x, gate_w)

    idx = [np.flatnonzero((top_i == e).any(axis=1)) for e in range(E)]
    wgt = []
    for e in range(E):
        sel = top_i[idx[e]] == e
        wgt.append(
            np.take_along_axis(top_v[idx[e]], np.argmax(sel, 1)[:, None], 1)[:, 0]
        )

    cmax = max(len(i) for i in idx)
    C = min(max(((cmax + 127) // 128) * 128, 128), C_CAP)
    n_dev = [min(len(i), C) for i in idx]

    if C not in _nc_cache:
        _nc_cache[C] = _build(C)
    nc = _nc_cache[C]

    in_maps = []
    for e in range(E):
        x_pad = np.zeros((C, H), dtype=np.float32)
        x_pad[: n_dev[e]] = x[idx[e][: n_dev[e]]]
        xb = np.ascontiguousarray(x_pad.T.reshape(KO, 128, C))
        t1 = w1[e].reshape(FT, 128, KO, 128)
        w1b = np.ascontiguousarray(t1.transpose(0, 3, 2, 1))
        t3 = w3[e].reshape(FT, 128, KO, 128)
        w3b = np.ascontiguousarray(t3.transpose(0, 3, 2, 1))
        w2b = np.ascontiguousarray(w2[e].T.reshape(FT, 128, H))
        in_maps.append({"xb": xb, "w1b": w1b, "w3b": w3b, "w2b": w2b})

    res = run_bass_kernel_spmd(nc, in_maps, core_ids=list(range(E)))

    out = np.zeros((T, H), dtype=np.float32)
    for e in range(E):
        y_e = res.results[e]["yb"]  # [C, H]
        out[idx[e][: n_dev[e]]] += wgt[e][: n_dev[e], None] * y_e[: n_dev[e]]
        if len(idx[e]) > n_dev[e]:
            # Overflow tokens past the capacity grid (a percent or so in the
            # worst-loaded expert): exact fp32 on host.
            xo = x[idx[e][n_dev[e] :]]
            h1 = xo @ w1[e].T
            a = (h1 / (1.0 + np.exp(-h1))) * (xo @ w3[e].T)
            yo = a @ w2[e].T
            out[idx[e][n_dev[e] :]] += wgt[e][n_dev[e] :, None] * yo
    return out.reshape(B, S, H)


# revision 13
# speedup vs baseline: 1.1820x; 1.0046x over previous
"""Mixtral sparse MoE block on 8 Trainium2 NeuronCores.

Expert-parallel: core e holds expert e's weights (w1/w3/w2 sharded on the E
axis), tokens are dispatched to cores by their top-2 expert assignment
(computed on host from the tiny replicated gate), each core runs the expert
GLU — y = (silu(x w1^T) * (x w3^T)) w2^T — over its token set in fp32r
(near-fp32 matmul accuracy at bf16 throughput), and the weighted combine is a
host-side scatter-add.

Device schedule, per core:
  Stage 1 keeps tokens in the matmul moving dim (chunks of 384, balancing the
  fp32r self-loading weight-load against the stream time) and produces
  actT [F, C] tiles in SBUF.  Stage 2 flips orientation: a 128-token slice of
  actT is the stationary operand and w2^T columns stream at N=512, so the
  output lands directly in [C, H] layout.  F is processed in two halves so
  the fp32r activation tensor fits in SBUF; the second half combines into the
  output with an accumulating DMA.
"""

import numpy as np

import concourse.mybir as mybir
import concourse.tile as tile
from concourse import bacc
from concourse.bass_utils import run_bass_kernel_spmd

H = 1024
F = 3584
E = 8
TOP_K = 2
KO = H // 128     # 8   k-tiles over H (stage-1 contraction)
HB = H // 512     # 2   h-blocks (stage-2 moving dim)
FT = F // 128     # 28  f-tiles over F
N_HALVES = 2
FH = FT // N_HALVES  # 14 f-tiles per half
S1_CHUNK = 512    # stage-1 moving-dim chunk
C_CAP = 1024      # device token capacity; overflow beyond this is tiny and
                  # computed on host (the fp32r weight-load floors matmul cost
                  # below N~390, so 2x512 chunks beat any 3-chunk split)

_nc_cache = {}


def _chunks(C, step):
    out = []
    off = 0
    while off < C:
        sz = min(step, C - off)
        out.append((off, sz))
        off += sz
    return out


def _build(C):
    f32r, f32 = mybir.dt.float32r, mybir.dt.float32
    s1_chunks = _chunks(C, S1_CHUNK)
    # f=0 only: small leading chunk so the first matmul group depends on a
    # ~1MB DMA instead of ~3MB; costs one extra (LDW-bound) group only once.
    if C > 256:
        s1_chunks_f0 = [(0, 128)] + _chunks(C - 128, S1_CHUNK)
        s1_chunks_f0 = [(0, 128)] + [(o + 128, s) for o, s in _chunks(C - 128, S1_CHUNK)]
    else:
        s1_chunks_f0 = s1_chunks
    TT = C // 128  # token tiles for stage 2

    nc = bacc.Bacc("TRN2", target_bir_lowering=False, debug=False, num_devices=E)
    xb = nc.dram_tensor("xb", [KO, 128, C], f32r, kind="ExternalInput")
    w1b = nc.dram_tensor("w1b", [FT, 128, KO, 128], f32r, kind="ExternalInput")
    w3b = nc.dram_tensor("w3b", [FT, 128, KO, 128], f32r, kind="ExternalInput")
    w2b = nc.dram_tensor("w2b", [FT, 128, H], f32r, kind="ExternalInput")
    yb = nc.dram_tensor("yb", [C, H], f32, kind="ExternalOutput")
    yb2 = nc.dram_tensor("yb2", [C, H], f32, kind="ExternalOutput")

    with tile.TileContext(nc) as tc:
        with (
            tc.tile_pool(name="xpool", bufs=1) as xpool,
            tc.tile_pool(name="actpool", bufs=1) as actpool,
            tc.tile_pool(name="w13pool", bufs=4) as w13pool,
            tc.tile_pool(name="w2pool", bufs=1) as w2pool,
            tc.tile_pool(name="outpool", bufs=4) as outpool,
            tc.tile_pool(name="silupool", bufs=4) as silupool,
            tc.tile_pool(name="ps1", bufs=2, space="PSUM") as ps1,
            tc.tile_pool(name="ps2", bufs=4, space="PSUM") as ps2,
        ):
            # Load the first f-tile's weights before everything else so the
            # PE can start as soon as the first x chunk lands.
            w1t0 = w13pool.tile([128, KO, 128], f32r, tag="w1t", name="w1t0")
            nc.sync.dma_start(w1t0[:], w1b[0])
            w3t0 = w13pool.tile([128, KO, 128], f32r, tag="w3t", name="w3t0")
            nc.sync.dma_start(w3t0[:], w3b[0])

            xt = xpool.tile([128, KO, C], f32r)
            for co, cs in s1_chunks_f0:
                nc.sync.dma_start(
                    xt[:, :, co : co + cs],
                    xb.rearrange("ko p c -> p ko c")[:, :, co : co + cs],
                )

            for half in range(N_HALVES):
                f0 = half * FH
                act = actpool.tile([128, FH, C], f32r, tag="act")

                # Stage 1: actT[f, c] = silu(w1 xT) * (w3 xT), per 128-row f tile
                for fi in range(FH):
                    f = f0 + fi
                    if f == 0:
                        w1t, w3t = w1t0, w3t0
                    else:
                        w1t = w13pool.tile([128, KO, 128], f32r, tag="w1t", name="w1t")
                        nc.sync.dma_start(w1t[:], w1b[f])
                        w3t = w13pool.tile([128, KO, 128], f32r, tag="w3t", name="w3t")
                        nc.sync.dma_start(w3t[:], w3b[f])
                    for co, cs in (s1_chunks_f0 if f == 0 else s1_chunks):
                        p1 = ps1.tile([128, S1_CHUNK], f32, tag="p1", name="p1")[:, :cs]
                        p3 = ps1.tile([128, S1_CHUNK], f32, tag="p3", name="p3")[:, :cs]
                        for ko in range(KO):
                            nc.tensor.matmul(
                                p1, w1t[:, ko], xt[:, ko, co : co + cs],
                                start=(ko == 0), stop=(ko == KO - 1),
                            )
                        for ko in range(KO):
                            nc.tensor.matmul(
                                p3, w3t[:, ko], xt[:, ko, co : co + cs],
                                start=(ko == 0), stop=(ko == KO - 1),
                            )
                        st = silupool.tile([128, S1_CHUNK], f32, tag="st", name="st")[:, :cs]
                        nc.scalar.activation(
                            st, p1, mybir.ActivationFunctionType.Silu
                        )
                        nc.vector.tensor_tensor(
                            act[:, fi, co : co + cs], st, p3, mybir.AluOpType.mult
                        )

                # w2 for this half: [128 ki, FH kf, H]  (issued after stage 1 so
                # its DMA doesn't delay the stage-1 weight stream; it overlaps
                # with stage-1 compute).
                w2t = w2pool.tile([128, FH, H], f32r, tag="w2t")
                nc.sync.dma_start(
                    w2t[:], w2b[f0 : f0 + FH].rearrange("kf p h -> p kf h")
                )

                # Stage 2: y[tok, h] += actT[:, tok-tile].T @ w2T[:, h-block]
                for t in range(TT):
                    ts = slice(t * 128, (t + 1) * 128)
                    for hb in range(HB):
                        hs = slice(hb * 512, (hb + 1) * 512)
                        py = ps2.tile([128, 512], f32, tag="py", name="py")
                        for kf in range(FH):
                            nc.tensor.matmul(
                                py, act[:, kf, ts], w2t[:, kf, hs],
                                start=(kf == 0), stop=(kf == FH - 1),
                            )
                        osb = outpool.tile([128, 512], f32, tag="osb", name="osb")
                        nc.vector.tensor_copy(osb[:], py[:])
                        if half == 0:
                            nc.sync.dma_start(yb[ts, hs], osb[:])
                        else:
                            nc.gpsimd.dma_start(yb2[ts, hs], osb[:])
    nc.compile()
    return nc


def _routing(x, gate_w):
    """Replicates the reference router in fp32 numpy: softmax over expert
    logits, top-2, renormalized weights.  Verified to match jax bit-for-bit
    on expert selection for these inputs (min top2/top3 prob gap 3e-5)."""
    logits = x @ gate_w.T
    m = logits.max(-1, keepdims=True)
    p = np.exp(logits - m)
    p /= p.sum(-1, keepdims=True)
    top_i = np.argsort(-p, axis=-1, kind="stable")[:, :TOP_K]
    top_v = np.take_along_axis(p, top_i, axis=-1)
    top_v = top_v / top_v.sum(-1, keepdims=True)
    return top_i, top_v


def kernel(hidden_states, gate_w, w1, w3, w2):
    B, S, _ = hidden_states.shape
    x = np.ascontiguousarray(
        np.asarray(hidden_states, dtype=np.float32).reshape(-1, H)
    )
    gate_w = np.asarray(gate_w, dtype=np.float32)
    w1 = np.asarray(w1, dtype=np.float32)
    w3 = np.asarray(w3, dtype=np.float32)
    w2 = np.asarray(w2, dtype=np.float32)
    T = x.shape[0]

    top_i, top_v = _routing(x, gate_w)

    idx = [np.flatnonzero((top_i == e).any(axis=1)) for e in range(E)]
    wgt = []
    for e in range(E):
        sel = top_i[idx[e]] == e
        wgt.append(
            np.take_along_axis(top_v[idx[e]], np.argmax(sel, 1)[:, None], 1)[:, 0]
        )

    cmax = max(len(i) for i in idx)
    C = min(max(((cmax + 127) // 128) * 128, 128), C_CAP)
    n_dev = [min(len(i), C) for i in idx]

    if C not in _nc_cache:
        _nc_cache[C] = _build(C)
    nc = _nc_cache[C]

    in_maps = []
    for e in range(E):
        x_pad = np.zeros((C, H), dtype=np.float32)
        x_pad[: n_dev[e]] = x[idx[e][: n_dev[e]]]
        xb = np.ascontiguousarray(x_pad.T.reshape(KO, 128, C))
        t1 = w1[e].reshape(FT, 128, KO, 128)
        w1b = np.ascontiguousarray(t1.transpose(0, 3, 2, 1))
        t3 = w3[e].reshape(FT, 128, KO, 128)
        w3b = np.ascontiguousarray(t3.transpose(0, 3, 2, 1))
        w2b = np.ascontiguousarray(w2[e].T.reshape(FT, 128, H))
        in_maps.append({"xb": xb, "w1b": w1b, "w3b": w3b, "w2b": w2b})

    res = run_bass_kernel_spmd(nc, in_maps, core_ids=list(range(E)))

    out = np.zeros((T, H), dtype=np.float32)
    for e in range(E):
        y_e = res.results[e]["yb"] + res.results[e]["yb2"]  # [C, H] halves
        out[idx[e][: n_dev[e]]] += wgt[e][: n_dev[e], None] * y_e[: n_dev[e]]
        if len(idx[e]) > n_dev[e]:
            # Overflow tokens past the capacity grid (a percent or so in the
            # worst-loaded expert): exact fp32 on host.
            xo = x[idx[e][n_dev[e] :]]
            h1 = xo @ w1[e].T
            a = (h1 / (1.0 + np.exp(-h1))) * (xo @ w3[e].T)
            yo = a @ w2[e].T
            out[idx[e][n_dev[e] :]] += wgt[e][n_dev[e] :, None] * yo
    return out.reshape(B, S, H)
